# revision 7
# baseline (speedup 1.0000x reference)
"""Bass/Tile TRN2 kernel for BiasMultiheadAttention (B=4, S=2048, D=512, H=8).

Sharding: one attention head per NeuronCore (8 heads / 8 cores). The attention
bias [1,H,S,S] is the dominant tensor (128 MB); head sharding loads each byte
of it exactly once (16 MB/core). The output projection mixes all heads, so it
runs as a second tiny NEFF, row-sharded across cores; the host only
transposes/slices inputs and concatenates outputs between phases.

Math layout per core (head h), all matmuls in float32r:
  QT = (SCALE*Wq_h) @ x^T + SCALE*bq   -> [64, B*S]   (dh on partitions)
  KT = Wk_h @ x^T + bk                 -> [64, B*S]
  V  = x @ Wv_h^T + bv                 -> [B*S, 64]   (stored per k-tile, with
                                            a ones column appended -> [128,65])
  S^T[k,q] = KT_tile^T @ QT_chunk      (PSUM, per batch)
  S^T += bias_h^T (DVE tensor add, bias host-transposed so tiles are [k,q])
  P^T = exp(S^T)                       (ACT, no max-subtraction: scores are O(1))
  O^T|sums = (V|1)^T @ P^T             (PSUM accum over k tiles -> [65, q])
  O^T norm = O^T * (1/sums) broadcast  (DVE recip + PE ones-broadcast + DVE mul)
Phase 2 (row-sharded): out = O^T^T @ w_out^T + b_out  (b_out via K=1 matmul).
"""

import sys

for _p in ("/opt/trn_rl_repo",):
    if _p not in sys.path:
        sys.path.append(_p)

import numpy as np

import concourse.bass as bass
import concourse.mybir as mybir
import concourse.tile as tile
from concourse import bacc
from concourse.bass_utils import run_bass_kernel_spmd

F32 = mybir.dt.float32
F32R = mybir.dt.float32r
EXPF = mybir.ActivationFunctionType.Exp
COPYF = mybir.ActivationFunctionType.Copy

N_CORES = 8
B, S, D = 4, 2048, 512
H, DH = 8, 64
SCALE = DH ** -0.5
ROWS = B * S            # 8192
RC = 512                # row chunk for projections
N_RC = ROWS // RC       # 16
FT = D // 128           # 4 feature tiles
KT_PER_B = S // 128     # 16 k-tiles per batch
QH = S // 2             # 1024, q processed in halves (bias SBUF residency)
QC = 512                # q chunk (one PSUM bank wide)
N_QC_H = QH // QC       # 2


def build_phase1():
    nc = bacc.Bacc("TRN2", target_bir_lowering=False, debug=False,
                   enable_asserts=False, num_devices=N_CORES)

    xT = nc.dram_tensor("xT", [D, ROWS], F32R, kind="ExternalInput")
    biasT = nc.dram_tensor("biasT", [S, S], F32, kind="ExternalInput")
    wqT = nc.dram_tensor("wqT", [D, DH], F32R, kind="ExternalInput")
    wkT = nc.dram_tensor("wkT", [D, DH], F32R, kind="ExternalInput")
    wvT = nc.dram_tensor("wvT", [D, DH], F32R, kind="ExternalInput")
    bq = nc.dram_tensor("bq", [1, DH], F32R, kind="ExternalInput")
    bk = nc.dram_tensor("bk", [1, DH], F32R, kind="ExternalInput")
    bv = nc.dram_tensor("bv", [1, DH], F32R, kind="ExternalInput")
    OT = nc.dram_tensor("OT", [DH, ROWS], F32, kind="ExternalOutput")

    with tile.TileContext(nc) as tc:
        with tc.tile_pool(name="persist", bufs=1) as persist:
            QT = persist.tile([DH, ROWS], F32R, tag="QT")
            KT = persist.tile([DH, ROWS], F32R, tag="KT")
            # V with ones column: [128, (b,kt), 65]
            Vaug = persist.tile([128, B * KT_PER_B, DH + 1], F32R, tag="Vaug")
            wq_sb = persist.tile([128, FT, DH], F32R, tag="wq")
            wk_sb = persist.tile([128, FT, DH], F32R, tag="wk")
            wv_sb = persist.tile([128, FT, DH], F32R, tag="wv")
            bq_sb = persist.tile([1, DH], F32R, tag="bq")
            bk_sb = persist.tile([1, DH], F32R, tag="bk")
            bv_sb = persist.tile([1, DH], F32R, tag="bv")
            ones = persist.tile([1, RC], F32R, tag="ones")
            # ones row living at partition DH(=64): lhsT for the sums
            # broadcast matmul, whose rhs (the recip row) is at partition 64.
            ones64 = persist.tile([DH + 1, 128], F32R, tag="ones64")

            nc.gpsimd.memset(ones[:].bitcast(F32), 1.0)
            nc.gpsimd.memset(ones64[DH:DH + 1, :].bitcast(F32), 1.0)
            nc.gpsimd.memset(Vaug[:, :, DH:DH + 1].bitcast(F32), 1.0)
            for w_sb, w_d in ((wq_sb, wqT), (wk_sb, wkT), (wv_sb, wvT)):
                nc.sync.dma_start(
                    w_sb[:], w_d.ap().rearrange("(t p) m -> p t m", p=128))
            for b_sb, b_d in ((bq_sb, bq), (bk_sb, bk), (bv_sb, bv)):
                nc.sync.dma_start(b_sb[:], b_d.ap())

            # ---------------- projections ----------------
            with tc.tile_pool(name="xtp", bufs=3) as xtp, \
                 tc.tile_pool(name="qk_ps", bufs=4, space="PSUM") as qk_ps, \
                 tc.tile_pool(name="v_ps", bufs=4, space="PSUM") as v_ps:
                for rc in range(N_RC):
                    xt = xtp.tile([128, FT, RC], F32R, tag="xt")
                    nc.sync.dma_start(
                        xt[:],
                        xT.ap()[:, rc * RC:(rc + 1) * RC]
                        .rearrange("(t p) r -> p t r", p=128))

                    for w_sb, b_sb, dst in ((wq_sb, bq_sb, QT), (wk_sb, bk_sb, KT)):
                        ps = qk_ps.tile([DH, RC], F32, tag="qk")
                        nc.tensor.matmul(ps[:], b_sb[:], ones[:],
                                         start=True, stop=False)
                        for ft in range(FT):
                            nc.tensor.matmul(ps[:], w_sb[:, ft, :], xt[:, ft, :],
                                             start=False, stop=(ft == FT - 1))
                        nc.scalar.copy(dst[:, rc * RC:(rc + 1) * RC], ps[:])

                    for sub in range(RC // 128):
                        ps = v_ps.tile([128, DH], F32, tag="v")
                        nc.tensor.matmul(ps[:], ones[:, 0:128], bv_sb[:],
                                         start=True, stop=False)
                        for ft in range(FT):
                            nc.tensor.matmul(
                                ps[:], xt[:, ft, sub * 128:(sub + 1) * 128],
                                wv_sb[:, ft, :],
                                start=False, stop=(ft == FT - 1))
                        rt = rc * (RC // 128) + sub
                        b_i, kt_i = divmod(rt, KT_PER_B)
                        nc.vector.tensor_copy(
                            Vaug[:, b_i * KT_PER_B + kt_i, 0:DH], ps[:])

            # ---------------- attention ----------------
            with tc.tile_pool(name="biasp", bufs=KT_PER_B + 1) as biasp, \
                 tc.tile_pool(name="ssb", bufs=2) as ssb, \
                 tc.tile_pool(name="esb", bufs=2) as esb, \
                 tc.tile_pool(name="osb", bufs=2) as osb, \
                 tc.tile_pool(name="onsb", bufs=2) as onsb, \
                 tc.tile_pool(name="sc_ps", bufs=2, space="PSUM") as sc_ps, \
                 tc.tile_pool(name="ot_ps", bufs=2, space="PSUM") as ot_ps, \
                 tc.tile_pool(name="bc_ps", bufs=1, space="PSUM") as bc_ps:
                for half in range(2):
                    q0 = half * QH
                    bias_tiles = []
                    for kt in range(KT_PER_B):
                        bt = biasp.tile([128, QH], F32, tag="bias")
                        nc.sync.dma_start(
                            bt[:], biasT.ap()[kt * 128:(kt + 1) * 128,
                                              q0:q0 + QH])
                        bias_tiles.append(bt)

                    for b_i in range(B):
                        qoff = b_i * S + q0
                        otps = [ot_ps.tile([DH + 1, QC], F32, tag="ot",
                                           name=f"ot_{half}_{b_i}_{qc}")
                                for qc in range(N_QC_H)]
                        for ktp in range(KT_PER_B // 2):
                            s_sb = ssb.tile([128, 2 * QH], F32, tag="s")
                            for j in range(2):
                                kt = 2 * ktp + j
                                koff = b_i * S + kt * 128
                                ps = sc_ps.tile([128, QH], F32, tag="sc")
                                for qc in range(N_QC_H):
                                    nc.tensor.matmul(
                                        ps[:, qc * QC:(qc + 1) * QC],
                                        KT[:, koff:koff + 128],
                                        QT[:, qoff + qc * QC:qoff + (qc + 1) * QC],
                                        start=True, stop=True)
                                nc.vector.tensor_add(
                                    s_sb[:, j * QH:(j + 1) * QH], ps[:],
                                    bias_tiles[kt][:])
                            e_sb = esb.tile([128, 2 * QH], F32R, tag="e")
                            nc.scalar.activation(e_sb[:], s_sb[:], EXPF)
                            for j in range(2):
                                kt = 2 * ktp + j
                                for qc in range(N_QC_H):
                                    nc.tensor.matmul(
                                        otps[qc][:],
                                        Vaug[:, b_i * KT_PER_B + kt, :],
                                        e_sb[:, j * QH + qc * QC:
                                             j * QH + (qc + 1) * QC],
                                        start=(ktp == 0 and j == 0),
                                        stop=(ktp == KT_PER_B // 2 - 1 and j == 1))

                        # normalize: O^T[:64] * (1/sums) ; sums = row 64
                        o_sb = osb.tile([DH + 1, QH], F32R, tag="o")
                        for qc in range(N_QC_H):
                            nc.scalar.copy(o_sb[:, qc * QC:(qc + 1) * QC],
                                           otps[qc][:])
                        with nc.allow_low_precision(
                                reason="softmax denom recip in f32r is fine"):
                            nc.vector.reciprocal(o_sb[DH:DH + 1, :],
                                                 o_sb[DH:DH + 1, :])
                        bc = bc_ps.tile([DH, QH], F32, tag="bc")
                        for qc in range(N_QC_H):
                            nc.tensor.matmul(
                                bc[:, qc * QC:(qc + 1) * QC],
                                ones64[DH:DH + 1, 0:DH],
                                o_sb[DH:DH + 1, qc * QC:(qc + 1) * QC],
                                start=True, stop=True)
                        on_sb = onsb.tile([DH, QH], F32, tag="on")
                        nc.vector.tensor_mul(on_sb[:], o_sb[0:DH, :], bc[:])
                        nc.sync.dma_start(OT.ap()[:, qoff:qoff + QH], on_sb[:])

    nc.compile()
    return nc


ROWS_PC = ROWS // N_CORES   # 1024 output rows per core in phase 2


def build_phase2():
    nc = bacc.Bacc("TRN2", target_bir_lowering=False, debug=False,
                   enable_asserts=False, num_devices=N_CORES)

    OTs = nc.dram_tensor("OTs", [D, ROWS_PC], F32R, kind="ExternalInput")
    woT = nc.dram_tensor("woT", [D, D], F32R, kind="ExternalInput")
    bo = nc.dram_tensor("bo", [1, D], F32R, kind="ExternalInput")
    out = nc.dram_tensor("out", [ROWS_PC, D], F32, kind="ExternalOutput")

    with tile.TileContext(nc) as tc:
        with tc.tile_pool(name="persist", bufs=1) as persist, \
             tc.tile_pool(name="res", bufs=3) as res, \
             tc.tile_pool(name="ps", bufs=4, space="PSUM") as psp:
            ot_sb = persist.tile([128, FT, ROWS_PC], F32R, tag="ot")
            wo_sb = persist.tile([128, FT, D], F32R, tag="wo")
            bo_sb = persist.tile([1, D], F32R, tag="bo")
            ones = persist.tile([1, 128], F32R, tag="ones")
            nc.gpsimd.memset(ones[:].bitcast(F32), 1.0)
            nc.sync.dma_start(ot_sb[:],
                              OTs.ap().rearrange("(t p) r -> p t r", p=128))
            nc.sync.dma_start(wo_sb[:],
                              woT.ap().rearrange("(t p) m -> p t m", p=128))
            nc.sync.dma_start(bo_sb[:], bo.ap())

            for rt in range(ROWS_PC // 128):
                ps = psp.tile([128, D], F32, tag="ps")
                nc.tensor.matmul(ps[:], ones[:], bo_sb[:],
                                 start=True, stop=False)
                for ft in range(FT):
                    nc.tensor.matmul(ps[:],
                                     ot_sb[:, ft, rt * 128:(rt + 1) * 128],
                                     wo_sb[:, ft, :],
                                     start=False, stop=(ft == FT - 1))
                r_sb = res.tile([128, D], F32, tag="r")
                nc.scalar.copy(r_sb[:], ps[:])
                nc.sync.dma_start(out.ap()[rt * 128:(rt + 1) * 128, :], r_sb[:])

    nc.compile()
    return nc


_CACHE = {}


def _get(name, builder):
    if name not in _CACHE:
        _CACHE[name] = builder()
    return _CACHE[name]


def kernel(x, attn_bias, w_in, b_in, w_out, b_out):
    x = np.asarray(x, dtype=np.float32)
    attn_bias = np.asarray(attn_bias, dtype=np.float32)
    w_in = np.asarray(w_in, dtype=np.float32)
    b_in = np.asarray(b_in, dtype=np.float32)
    w_out = np.asarray(w_out, dtype=np.float32)
    b_out = np.asarray(b_out, dtype=np.float32)

    nc1 = _get("p1", build_phase1)
    nc2 = _get("p2", build_phase2)

    xT = np.ascontiguousarray(x.reshape(ROWS, D).T)
    in_maps1 = []
    for h in range(N_CORES):
        sl_q = slice(h * DH, (h + 1) * DH)
        in_maps1.append({
            "xT": xT,
            "biasT": np.ascontiguousarray(attn_bias[0, h].T),
            "wqT": np.ascontiguousarray(w_in[sl_q, :].T) * SCALE,
            "wkT": np.ascontiguousarray(w_in[D + h * DH:D + (h + 1) * DH, :].T),
            "wvT": np.ascontiguousarray(
                w_in[2 * D + h * DH:2 * D + (h + 1) * DH, :].T),
            "bq": (b_in[sl_q] * SCALE).reshape(1, DH).copy(),
            "bk": b_in[D + h * DH:D + (h + 1) * DH].reshape(1, DH).copy(),
            "bv": b_in[2 * D + h * DH:2 * D + (h + 1) * DH].reshape(1, DH).copy(),
        })
    res1 = run_bass_kernel_spmd(nc1, in_maps1, core_ids=list(range(N_CORES)))
    OT_full = np.concatenate([res1.results[h]["OT"] for h in range(N_CORES)],
                             axis=0)  # [512, 8192]

    woT = np.ascontiguousarray(w_out.T)
    bo = b_out.reshape(1, D).copy()
    in_maps2 = [{
        "OTs": np.ascontiguousarray(
            OT_full[:, r * ROWS_PC:(r + 1) * ROWS_PC]),
        "woT": woT,
        "bo": bo,
    } for r in range(N_CORES)]
    res2 = run_bass_kernel_spmd(nc2, in_maps2, core_ids=list(range(N_CORES)))
    out = np.concatenate([res2.results[r]["out"] for r in range(N_CORES)],
                         axis=0)
    return out.reshape(B, S, D)


# revision 11
# speedup vs baseline: 1.0891x; 1.0891x over previous
"""Bass/Tile TRN2 kernel for BiasMultiheadAttention (B=4, S=2048, D=512, H=8).

Sharding: one attention head per NeuronCore (8 heads / 8 cores). The attention
bias [1,H,S,S] is the dominant tensor (128 MB); head sharding loads each byte
of it exactly once (16 MB/core). The output projection mixes all heads, so it
runs as a second tiny NEFF, row-sharded across cores; the host only
transposes/slices inputs and concatenates outputs between phases.

Math layout per core (head h), all matmuls in float32r:
  QT = (SCALE*Wq_h) @ x^T + SCALE*bq   -> [64, B*S]   (dh on partitions)
  KT = Wk_h @ x^T + bk                 -> [64, B*S]
  V  = x @ Wv_h^T + bv                 -> [B*S, 64]   (stored per k-tile, with
                                            a ones column appended -> [128,65])
  S^T[k,q] = KT_tile^T @ QT_chunk      (PSUM, per batch)
  S^T += bias_h^T (DVE tensor add, bias host-transposed so tiles are [k,q])
  P^T = exp(S^T)                       (ACT, no max-subtraction: scores are O(1))
  O^T|sums = (V|1)^T @ P^T             (PSUM accum over k tiles -> [65, q])
  O^T norm = O^T * (1/sums) broadcast  (DVE recip + PE ones-broadcast + DVE mul)
Phase 2 (row-sharded): out = O^T^T @ w_out^T + b_out  (b_out via K=1 matmul).
"""

import sys

for _p in ("/opt/trn_rl_repo",):
    if _p not in sys.path:
        sys.path.append(_p)

import numpy as np

import concourse.bass as bass
import concourse.mybir as mybir
import concourse.tile as tile
from concourse import bacc
from concourse.bass_utils import run_bass_kernel_spmd

F32 = mybir.dt.float32
F32R = mybir.dt.float32r
EXPF = mybir.ActivationFunctionType.Exp
COPYF = mybir.ActivationFunctionType.Copy

N_CORES = 8
B, S, D = 4, 2048, 512
H, DH = 8, 64
SCALE = DH ** -0.5
ROWS = B * S            # 8192
RC = 512                # row chunk for projections
N_RC = ROWS // RC       # 16
FT = D // 128           # 4 feature tiles
KT_PER_B = S // 128     # 16 k-tiles per batch
QH = S // 2             # 1024, q processed in halves (bias SBUF residency)
QC = 512                # q chunk (one PSUM bank wide)
N_QC_H = QH // QC       # 2


def build_phase1(reps=1, ablate=()):
    nc = bacc.Bacc("TRN2", target_bir_lowering=False, debug=False,
                   enable_asserts=False, num_devices=N_CORES)

    xT = nc.dram_tensor("xT", [D, ROWS], F32R, kind="ExternalInput")
    biasT = nc.dram_tensor("biasT", [S, S], F32, kind="ExternalInput")
    wqT = nc.dram_tensor("wqT", [D, DH], F32R, kind="ExternalInput")
    wkT = nc.dram_tensor("wkT", [D, DH], F32R, kind="ExternalInput")
    wvT = nc.dram_tensor("wvT", [D, DH], F32R, kind="ExternalInput")
    bq = nc.dram_tensor("bq", [DH, 1], F32, kind="ExternalInput")
    bk = nc.dram_tensor("bk", [DH, 1], F32, kind="ExternalInput")
    bv = nc.dram_tensor("bv", [1, DH], F32R, kind="ExternalInput")
    OT = nc.dram_tensor("OT", [DH, ROWS], F32, kind="ExternalOutput")

    with tile.TileContext(nc) as tc:
        with tc.tile_pool(name="persist", bufs=1) as persist:
            QT = persist.tile([DH, ROWS], F32R, tag="QT")
            KT = persist.tile([DH, ROWS], F32R, tag="KT")
            # V with ones column: [128, (b,kt), 65]
            Vaug = persist.tile([128, B * KT_PER_B, DH + 1], F32R, tag="Vaug")
            wq_sb = persist.tile([128, FT, DH], F32R, tag="wq")
            wk_sb = persist.tile([128, FT, DH], F32R, tag="wk")
            wv_sb = persist.tile([128, FT, DH], F32R, tag="wv")
            bq_sb = persist.tile([DH, 1], F32, tag="bq")
            bk_sb = persist.tile([DH, 1], F32, tag="bk")
            bv_sb = persist.tile([1, DH], F32R, tag="bv")
            ones = persist.tile([1, RC], F32R, tag="ones")
            # ones row living at partition DH(=64): lhsT for the sums
            # broadcast matmul, whose rhs (the recip row) is at partition 64.
            ones64 = persist.tile([DH + 1, 128], F32R, tag="ones64")

            nc.gpsimd.memset(ones[:].bitcast(F32), 1.0)
            nc.gpsimd.memset(ones64[DH:DH + 1, :].bitcast(F32), 1.0)
            nc.gpsimd.memset(Vaug[:, :, DH:DH + 1].bitcast(F32), 1.0)
            for w_sb, w_d in ((wq_sb, wqT), (wk_sb, wkT), (wv_sb, wvT)):
                nc.sync.dma_start(
                    w_sb[:], w_d.ap().rearrange("(t p) m -> p t m", p=128))
            for b_sb, b_d in ((bq_sb, bq), (bk_sb, bk), (bv_sb, bv)):
                nc.sync.dma_start(b_sb[:], b_d.ap())

            # ---------------- body (optionally repeated for timing) ----
            import contextlib

            def body():
                run_body(nc, tc, locals_ns)

            locals_ns = dict(QT=QT, KT=KT, Vaug=Vaug, wq_sb=wq_sb,
                             wk_sb=wk_sb, wv_sb=wv_sb, bq_sb=bq_sb,
                             bk_sb=bk_sb, bv_sb=bv_sb, ones=ones,
                             ones64=ones64, xT=xT, biasT=biasT, OT=OT,
                             ablate=ablate)
            if reps == 1:
                body()
            else:
                with tc.For_i(0, reps, 1):
                    body()

    nc.compile()
    return nc


def run_body(nc, tc, ns):
    QT, KT, Vaug = ns["QT"], ns["KT"], ns["Vaug"]
    wq_sb, wk_sb, wv_sb = ns["wq_sb"], ns["wk_sb"], ns["wv_sb"]
    bq_sb, bk_sb, bv_sb = ns["bq_sb"], ns["bk_sb"], ns["bv_sb"]
    ones, ones64 = ns["ones"], ns["ones64"]
    xT, biasT, OT = ns["xT"], ns["biasT"], ns["OT"]
    ablate = ns.get("ablate", ())
    if True:
        if True:
            # ---------------- projections ----------------
            with tc.tile_pool(name="xtp", bufs=3) as xtp, \
                 tc.tile_pool(name="qk_ps", bufs=4, space="PSUM") as qk_ps, \
                 tc.tile_pool(name="v_ps", bufs=4, space="PSUM") as v_ps:
                for rc in range(N_RC):
                    xt = xtp.tile([128, FT, RC], F32R, tag="xt")
                    nc.sync.dma_start(
                        xt[:],
                        xT.ap()[:, rc * RC:(rc + 1) * RC]
                        .rearrange("(t p) r -> p t r", p=128))

                    for w_sb, b_sb, dst in ((wq_sb, bq_sb, QT), (wk_sb, bk_sb, KT)):
                        ps = qk_ps.tile([DH, RC], F32, tag="qk")
                        for ft in range(FT):
                            nc.tensor.matmul(ps[:], w_sb[:, ft, :], xt[:, ft, :],
                                             start=(ft == 0), stop=(ft == FT - 1))
                        nc.scalar.activation(
                            dst[:, rc * RC:(rc + 1) * RC], ps[:],
                            mybir.ActivationFunctionType.Identity,
                            bias=b_sb[:])

                    for sub in range(RC // 128):
                        ps = v_ps.tile([128, DH], F32, tag="v")
                        nc.tensor.matmul(ps[:], ones[:, 0:128], bv_sb[:],
                                         start=True, stop=False)
                        for ft in range(FT):
                            nc.tensor.matmul(
                                ps[:], xt[:, ft, sub * 128:(sub + 1) * 128],
                                wv_sb[:, ft, :],
                                start=False, stop=(ft == FT - 1))
                        rt = rc * (RC // 128) + sub
                        b_i, kt_i = divmod(rt, KT_PER_B)
                        nc.scalar.copy(
                            Vaug[:, b_i * KT_PER_B + kt_i, 0:DH], ps[:])

            # ---------------- attention ----------------
            with tc.tile_pool(name="biasp", bufs=KT_PER_B + 1) as biasp, \
                 tc.tile_pool(name="ssb", bufs=3) as ssb, \
                 tc.tile_pool(name="osb", bufs=2) as osb, \
                 tc.tile_pool(name="onsb", bufs=2) as onsb, \
                 tc.tile_pool(name="sc_ps", bufs=2, space="PSUM") as sc_ps, \
                 tc.tile_pool(name="ot_ps", bufs=2, space="PSUM") as ot_ps, \
                 tc.tile_pool(name="bc_ps", bufs=1, space="PSUM") as bc_ps:
                for half in range(2):
                    q0 = half * QH
                    bias_tiles = []
                    for kt in range(KT_PER_B):
                        bt = biasp.tile([128, QH], F32, tag="bias")
                        nc.sync.dma_start(
                            bt[:], biasT.ap()[kt * 128:(kt + 1) * 128,
                                              q0:q0 + QH])
                        bias_tiles.append(bt)

                    for b_i in range(B):
                        qoff = b_i * S + q0
                        otps = [ot_ps.tile([DH + 1, QC], F32, tag="ot",
                                           name=f"ot_{half}_{b_i}_{qc}")
                                for qc in range(N_QC_H)]
                        for ktp in range(KT_PER_B // 2):
                            s_sb = ssb.tile([128, 2 * QH], F32R, tag="s")
                            for j in range(2):
                                kt = 2 * ktp + j
                                koff = b_i * S + kt * 128
                                ps = sc_ps.tile([128, QH], F32, tag="sc")
                                for qc in range(N_QC_H):
                                    nc.tensor.matmul(
                                        ps[:, qc * QC:(qc + 1) * QC],
                                        KT[:, koff:koff + 128],
                                        QT[:, qoff + qc * QC:qoff + (qc + 1) * QC],
                                        start=True, stop=True)
                                if "tt" in ablate:
                                    nc.vector.tensor_copy(
                                        s_sb[:, j * QH:(j + 1) * QH], ps[:])
                                else:
                                    nc.vector.tensor_add(
                                        s_sb[:, j * QH:(j + 1) * QH], ps[:],
                                        bias_tiles[kt][:])
                            if "exp" not in ablate:
                                nc.scalar.activation(s_sb[:], s_sb[:], EXPF)
                            if "av" not in ablate:
                                for j in range(2):
                                    kt = 2 * ktp + j
                                    for qc in range(N_QC_H):
                                        nc.tensor.matmul(
                                            otps[qc][:],
                                            Vaug[:, b_i * KT_PER_B + kt, :],
                                            s_sb[:, j * QH + qc * QC:
                                                 j * QH + (qc + 1) * QC],
                                            start=(ktp == 0 and j == 0),
                                            stop=(ktp == KT_PER_B // 2 - 1
                                                  and j == 1))

                        if "av" in ablate:
                            continue
                        # normalize: O^T[:64] * (1/sums) ; sums = row 64
                        o_sb = osb.tile([DH + 1, QH], F32R, tag="o")
                        for qc in range(N_QC_H):
                            nc.scalar.copy(o_sb[:, qc * QC:(qc + 1) * QC],
                                           otps[qc][:])
                        with nc.allow_low_precision(
                                reason="softmax denom recip in f32r is fine"):
                            nc.vector.reciprocal(o_sb[DH:DH + 1, :],
                                                 o_sb[DH:DH + 1, :])
                        bc = bc_ps.tile([DH, QH], F32, tag="bc")
                        for qc in range(N_QC_H):
                            nc.tensor.matmul(
                                bc[:, qc * QC:(qc + 1) * QC],
                                ones64[DH:DH + 1, 0:DH],
                                o_sb[DH:DH + 1, qc * QC:(qc + 1) * QC],
                                start=True, stop=True)
                        on_sb = onsb.tile([DH, QH], F32, tag="on")
                        nc.vector.tensor_mul(on_sb[:], o_sb[0:DH, :], bc[:])
                        nc.sync.dma_start(OT.ap()[:, qoff:qoff + QH], on_sb[:])


ROWS_PC = ROWS // N_CORES   # 1024 output rows per core in phase 2


def build_phase2(reps=1):
    nc = bacc.Bacc("TRN2", target_bir_lowering=False, debug=False,
                   enable_asserts=False, num_devices=N_CORES)

    OTs = nc.dram_tensor("OTs", [D, ROWS_PC], F32R, kind="ExternalInput")
    woT = nc.dram_tensor("woT", [D, D], F32R, kind="ExternalInput")
    bo = nc.dram_tensor("bo", [1, D], F32R, kind="ExternalInput")
    out = nc.dram_tensor("out", [ROWS_PC, D], F32, kind="ExternalOutput")

    with tile.TileContext(nc) as tc:
        with tc.tile_pool(name="persist", bufs=1) as persist, \
             tc.tile_pool(name="res", bufs=3) as res, \
             tc.tile_pool(name="ps", bufs=4, space="PSUM") as psp:
            ot_sb = persist.tile([128, FT, ROWS_PC], F32R, tag="ot")
            wo_sb = persist.tile([128, FT, D], F32R, tag="wo")
            bo_sb = persist.tile([1, D], F32R, tag="bo")
            ones = persist.tile([1, 128], F32R, tag="ones")
            nc.gpsimd.memset(ones[:].bitcast(F32), 1.0)
            nc.sync.dma_start(wo_sb[:],
                              woT.ap().rearrange("(t p) m -> p t m", p=128))
            nc.sync.dma_start(bo_sb[:], bo.ap())

            def p2_body():
                for rt in range(ROWS_PC // 128):
                    nc.sync.dma_start(
                        ot_sb[:, :, rt * 128:(rt + 1) * 128],
                        OTs.ap()[:, rt * 128:(rt + 1) * 128]
                        .rearrange("(t p) r -> p t r", p=128))
                    ps = psp.tile([128, D], F32, tag="ps")
                    nc.tensor.matmul(ps[:], ones[:], bo_sb[:],
                                     start=True, stop=False)
                    for ft in range(FT):
                        nc.tensor.matmul(
                            ps[:], ot_sb[:, ft, rt * 128:(rt + 1) * 128],
                            wo_sb[:, ft, :],
                            start=False, stop=(ft == FT - 1))
                    r_sb = res.tile([128, D], F32, tag="r")
                    nc.scalar.copy(r_sb[:], ps[:])
                    nc.sync.dma_start(out.ap()[rt * 128:(rt + 1) * 128, :],
                                      r_sb[:])

            if reps == 1:
                p2_body()
            else:
                with tc.For_i(0, reps, 1):
                    p2_body()

    nc.compile()
    return nc


_CACHE = {}


def _get(name, builder):
    if name not in _CACHE:
        _CACHE[name] = builder()
    return _CACHE[name]


def kernel(x, attn_bias, w_in, b_in, w_out, b_out):
    x = np.asarray(x, dtype=np.float32)
    attn_bias = np.asarray(attn_bias, dtype=np.float32)
    w_in = np.asarray(w_in, dtype=np.float32)
    b_in = np.asarray(b_in, dtype=np.float32)
    w_out = np.asarray(w_out, dtype=np.float32)
    b_out = np.asarray(b_out, dtype=np.float32)

    nc1 = _get("p1", build_phase1)
    nc2 = _get("p2", build_phase2)

    xT = np.ascontiguousarray(x.reshape(ROWS, D).T)
    in_maps1 = []
    for h in range(N_CORES):
        sl_q = slice(h * DH, (h + 1) * DH)
        in_maps1.append({
            "xT": xT,
            "biasT": np.ascontiguousarray(attn_bias[0, h].T),
            "wqT": np.ascontiguousarray(w_in[sl_q, :].T) * SCALE,
            "wkT": np.ascontiguousarray(w_in[D + h * DH:D + (h + 1) * DH, :].T),
            "wvT": np.ascontiguousarray(
                w_in[2 * D + h * DH:2 * D + (h + 1) * DH, :].T),
            "bq": (b_in[sl_q] * SCALE).reshape(DH, 1).copy(),
            "bk": b_in[D + h * DH:D + (h + 1) * DH].reshape(DH, 1).copy(),
            "bv": b_in[2 * D + h * DH:2 * D + (h + 1) * DH].reshape(1, DH).copy(),
        })
    res1 = run_bass_kernel_spmd(nc1, in_maps1, core_ids=list(range(N_CORES)))
    OT_full = np.concatenate([res1.results[h]["OT"] for h in range(N_CORES)],
                             axis=0)  # [512, 8192]

    woT = np.ascontiguousarray(w_out.T)
    bo = b_out.reshape(1, D).copy()
    in_maps2 = [{
        "OTs": np.ascontiguousarray(
            OT_full[:, r * ROWS_PC:(r + 1) * ROWS_PC]),
        "woT": woT,
        "bo": bo,
    } for r in range(N_CORES)]
    res2 = run_bass_kernel_spmd(nc2, in_maps2, core_ids=list(range(N_CORES)))
    out = np.concatenate([res2.results[r]["out"] for r in range(N_CORES)],
                         axis=0)
    return out.reshape(B, S, D)


# revision 12
# speedup vs baseline: 1.1467x; 1.0528x over previous
"""Bass/Tile TRN2 kernel for BiasMultiheadAttention (B=4, S=2048, D=512, H=8).

Sharding: one attention head per NeuronCore (8 heads / 8 cores). The attention
bias [1,H,S,S] is the dominant tensor (128 MB); head sharding loads each byte
of it exactly once (16 MB/core). The output projection mixes all heads, so it
runs as a second tiny NEFF, row-sharded across cores; the host only
transposes/slices inputs and concatenates outputs between phases.

Math layout per core (head h), all matmuls in float32r:
  QT = (SCALE*Wq_h) @ x^T + SCALE*bq   -> [64, B*S]   (dh on partitions)
  KT = Wk_h @ x^T + bk                 -> [64, B*S]
  V  = x @ Wv_h^T + bv                 -> [B*S, 64]   (stored per k-tile, with
                                            a ones column appended -> [128,65])
  S^T[k,q] = KT_tile^T @ QT_chunk      (PSUM, per batch)
  S^T += bias_h^T (DVE tensor add, bias host-transposed so tiles are [k,q])
  P^T = exp(S^T)                       (ACT, no max-subtraction: scores are O(1))
  O^T|sums = (V|1)^T @ P^T             (PSUM accum over k tiles -> [65, q])
  O^T norm = O^T * (1/sums) broadcast  (DVE recip + PE ones-broadcast + DVE mul)
Phase 2 (row-sharded): out = O^T^T @ w_out^T + b_out  (b_out via K=1 matmul).
"""

import sys

for _p in ("/opt/trn_rl_repo",):
    if _p not in sys.path:
        sys.path.append(_p)

import numpy as np

import concourse.bass as bass
import concourse.mybir as mybir
import concourse.tile as tile
from concourse import bacc
from concourse.bass_utils import run_bass_kernel_spmd

F32 = mybir.dt.float32
F32R = mybir.dt.float32r
EXPF = mybir.ActivationFunctionType.Exp
COPYF = mybir.ActivationFunctionType.Copy

N_CORES = 8
B, S, D = 4, 2048, 512
H, DH = 8, 64
SCALE = DH ** -0.5
ROWS = B * S            # 8192
RC = 512                # row chunk for projections
N_RC = ROWS // RC       # 16
FT = D // 128           # 4 feature tiles
KT_PER_B = S // 128     # 16 k-tiles per batch
QH = S // 2             # 1024, q processed in halves (bias SBUF residency)
QC = 512                # q chunk (one PSUM bank wide)
N_QC_H = QH // QC       # 2


def build_phase1(reps=1, ablate=()):
    nc = bacc.Bacc("TRN2", target_bir_lowering=False, debug=False,
                   enable_asserts=False, num_devices=N_CORES)

    xT = nc.dram_tensor("xT", [D, ROWS], F32R, kind="ExternalInput")
    biasT = nc.dram_tensor("biasT", [S, S], F32, kind="ExternalInput")
    wqkT = nc.dram_tensor("wqkT", [D, 2 * DH], F32R, kind="ExternalInput")
    wvT = nc.dram_tensor("wvT", [D, DH], F32R, kind="ExternalInput")
    bqk = nc.dram_tensor("bqk", [2 * DH, 1], F32, kind="ExternalInput")
    bv = nc.dram_tensor("bv", [1, DH], F32R, kind="ExternalInput")
    OT = nc.dram_tensor("OT", [DH, ROWS], F32, kind="ExternalOutput")

    with tile.TileContext(nc) as tc:
        with tc.tile_pool(name="persist", bufs=1) as persist:
            QKT = persist.tile([2 * DH, ROWS], F32R, tag="QKT")
            KTx = persist.tile([DH, ROWS], F32R, tag="KTx")
            # V with ones column: [128, (b,kt), 65]
            Vaug = persist.tile([128, B * KT_PER_B, DH + 1], F32R, tag="Vaug")
            wqk_sb = persist.tile([128, FT, 2 * DH], F32R, tag="wqk")
            wv_sb = persist.tile([128, FT, DH], F32R, tag="wv")
            bqk_sb = persist.tile([2 * DH, 1], F32, tag="bqk")
            bv_sb = persist.tile([1, DH], F32R, tag="bv")
            ones = persist.tile([1, RC], F32R, tag="ones")
            # ones row living at partition DH(=64): lhsT for the sums
            # broadcast matmul, whose rhs (the recip row) is at partition 64.
            ones64 = persist.tile([DH + 1, 128], F32R, tag="ones64")

            nc.gpsimd.memset(ones[:].bitcast(F32), 1.0)
            nc.gpsimd.memset(ones64[DH:DH + 1, :].bitcast(F32), 1.0)
            nc.gpsimd.memset(Vaug[:, :, DH:DH + 1].bitcast(F32), 1.0)
            for w_sb, w_d in ((wqk_sb, wqkT), (wv_sb, wvT)):
                nc.sync.dma_start(
                    w_sb[:], w_d.ap().rearrange("(t p) m -> p t m", p=128))
            for b_sb, b_d in ((bqk_sb, bqk), (bv_sb, bv)):
                nc.sync.dma_start(b_sb[:], b_d.ap())

            # ---------------- body (optionally repeated for timing) ----
            import contextlib

            def body():
                run_body(nc, tc, locals_ns)

            locals_ns = dict(QKT=QKT, KTx=KTx, Vaug=Vaug, wqk_sb=wqk_sb,
                             wv_sb=wv_sb, bqk_sb=bqk_sb, bv_sb=bv_sb,
                             ones=ones, ones64=ones64, xT=xT, biasT=biasT,
                             OT=OT, ablate=ablate)
            if reps == 1:
                body()
            else:
                with tc.For_i(0, reps, 1):
                    body()

    nc.compile()
    return nc


def run_body(nc, tc, ns):
    QKT, KTx, Vaug = ns["QKT"], ns["KTx"], ns["Vaug"]
    wqk_sb, wv_sb = ns["wqk_sb"], ns["wv_sb"]
    bqk_sb, bv_sb = ns["bqk_sb"], ns["bv_sb"]
    ones, ones64 = ns["ones"], ns["ones64"]
    xT, biasT, OT = ns["xT"], ns["biasT"], ns["OT"]
    ablate = ns.get("ablate", ())
    if True:
        if True:
            # ---------------- projections ----------------
            with tc.tile_pool(name="xtp", bufs=2) as xtp, \
                 tc.tile_pool(name="qk_ps", bufs=4, space="PSUM") as qk_ps, \
                 tc.tile_pool(name="v_ps", bufs=4, space="PSUM") as v_ps:
                for rc in range(N_RC):
                    xt = xtp.tile([128, FT, RC], F32R, tag="xt")
                    nc.sync.dma_start(
                        xt[:],
                        xT.ap()[:, rc * RC:(rc + 1) * RC]
                        .rearrange("(t p) r -> p t r", p=128))

                    ps = qk_ps.tile([2 * DH, RC], F32, tag="qk")
                    for ft in range(FT):
                        nc.tensor.matmul(ps[:], wqk_sb[:, ft, :], xt[:, ft, :],
                                         start=(ft == 0), stop=(ft == FT - 1))
                    nc.scalar.activation(
                        QKT[:, rc * RC:(rc + 1) * RC], ps[:],
                        mybir.ActivationFunctionType.Identity,
                        bias=bqk_sb[:])
                    nc.sync.dma_start(
                        KTx[:, rc * RC:(rc + 1) * RC],
                        QKT[DH:2 * DH, rc * RC:(rc + 1) * RC])

                    for sub in range(RC // 128):
                        ps = v_ps.tile([128, DH], F32, tag="v")
                        nc.tensor.matmul(ps[:], ones[:, 0:128], bv_sb[:],
                                         start=True, stop=False)
                        for ft in range(FT):
                            nc.tensor.matmul(
                                ps[:], xt[:, ft, sub * 128:(sub + 1) * 128],
                                wv_sb[:, ft, :],
                                start=False, stop=(ft == FT - 1))
                        rt = rc * (RC // 128) + sub
                        b_i, kt_i = divmod(rt, KT_PER_B)
                        nc.scalar.copy(
                            Vaug[:, b_i * KT_PER_B + kt_i, 0:DH], ps[:])

            # ---------------- attention ----------------
            with tc.tile_pool(name="biasp", bufs=KT_PER_B + 1) as biasp, \
                 tc.tile_pool(name="ssb", bufs=2) as ssb, \
                 tc.tile_pool(name="esb", bufs=2) as esb, \
                 tc.tile_pool(name="osb", bufs=2) as osb, \
                 tc.tile_pool(name="onsb", bufs=2) as onsb, \
                 tc.tile_pool(name="sc_ps", bufs=3, space="PSUM") as sc_ps, \
                 tc.tile_pool(name="ot_ps", bufs=2, space="PSUM") as ot_ps:
                for half in range(2):
                    q0 = half * QH
                    bias_tiles = []
                    for kt in range(KT_PER_B):
                        bt = biasp.tile([128, QH], F32, tag="bias")
                        nc.sync.dma_start(
                            bt[:], biasT.ap()[kt * 128:(kt + 1) * 128,
                                              q0:q0 + QH])
                        bias_tiles.append(bt)

                    for b_i in range(B):
                        qoff = b_i * S + q0
                        otps = [ot_ps.tile([DH + 1, QC], F32, tag="ot",
                                           name=f"ot_{half}_{b_i}_{qc}")
                                for qc in range(N_QC_H)]
                        for ktp in range(KT_PER_B // 2):
                            s_sb = ssb.tile([128, 2 * QH], F32, tag="s")
                            for j in range(2):
                                kt = 2 * ktp + j
                                koff = b_i * S + kt * 128
                                ps = sc_ps.tile([128, QH], F32, tag="sc")
                                for qc in range(N_QC_H):
                                    nc.tensor.matmul(
                                        ps[:, qc * QC:(qc + 1) * QC],
                                        KTx[:, koff:koff + 128],
                                        QKT[0:DH, qoff + qc * QC:
                                            qoff + (qc + 1) * QC],
                                        start=True, stop=True)
                                if "tt" in ablate:
                                    nc.vector.tensor_copy(
                                        s_sb[:, j * QH:(j + 1) * QH], ps[:])
                                else:
                                    nc.vector.tensor_add(
                                        s_sb[:, j * QH:(j + 1) * QH], ps[:],
                                        bias_tiles[kt][:])
                            e_sb = esb.tile([128, 2 * QH], F32R, tag="e")
                            if "exp" not in ablate:
                                nc.scalar.activation(e_sb[:], s_sb[:], EXPF)
                            else:
                                nc.scalar.copy(e_sb[:], s_sb[:])
                            if "av" not in ablate:
                                for j in range(2):
                                    kt = 2 * ktp + j
                                    for qc in range(N_QC_H):
                                        nc.tensor.matmul(
                                            otps[qc][:],
                                            Vaug[:, b_i * KT_PER_B + kt, :],
                                            e_sb[:, j * QH + qc * QC:
                                                 j * QH + (qc + 1) * QC],
                                            start=(ktp == 0 and j == 0),
                                            stop=(ktp == KT_PER_B // 2 - 1
                                                  and j == 1))

                        if "av" in ablate:
                            continue
                        # normalize: O^T[:64] * (1/sums) ; sums = row 64
                        o_sb = osb.tile([DH + 1, QH], F32R, tag="o")
                        for qc in range(N_QC_H):
                            nc.scalar.copy(o_sb[:, qc * QC:(qc + 1) * QC],
                                           otps[qc][:])
                        with nc.allow_low_precision(
                                reason="softmax denom recip in f32r is fine"):
                            nc.vector.reciprocal(o_sb[DH:DH + 1, :],
                                                 o_sb[DH:DH + 1, :])
                        bc = sc_ps.tile([DH, QH], F32, tag="sc", name="bc")
                        for qc in range(N_QC_H):
                            nc.tensor.matmul(
                                bc[:, qc * QC:(qc + 1) * QC],
                                ones64[DH:DH + 1, 0:DH],
                                o_sb[DH:DH + 1, qc * QC:(qc + 1) * QC],
                                start=True, stop=True)
                        on_sb = onsb.tile([DH, QH], F32, tag="on")
                        nc.vector.tensor_mul(on_sb[:], o_sb[0:DH, :], bc[:])
                        nc.sync.dma_start(OT.ap()[:, qoff:qoff + QH], on_sb[:])


ROWS_PC = ROWS // N_CORES   # 1024 output rows per core in phase 2


def build_phase2(reps=1):
    nc = bacc.Bacc("TRN2", target_bir_lowering=False, debug=False,
                   enable_asserts=False, num_devices=N_CORES)

    OTs = nc.dram_tensor("OTs", [D, ROWS_PC], F32R, kind="ExternalInput")
    woT = nc.dram_tensor("woT", [D, D], F32R, kind="ExternalInput")
    bo = nc.dram_tensor("bo", [1, D], F32R, kind="ExternalInput")
    out = nc.dram_tensor("out", [ROWS_PC, D], F32, kind="ExternalOutput")

    with tile.TileContext(nc) as tc:
        with tc.tile_pool(name="persist", bufs=1) as persist, \
             tc.tile_pool(name="res", bufs=3) as res, \
             tc.tile_pool(name="ps", bufs=4, space="PSUM") as psp:
            ot_sb = persist.tile([128, FT, ROWS_PC], F32R, tag="ot")
            wo_sb = persist.tile([128, FT, D], F32R, tag="wo")
            bo_sb = persist.tile([1, D], F32R, tag="bo")
            ones = persist.tile([1, 128], F32R, tag="ones")
            nc.gpsimd.memset(ones[:].bitcast(F32), 1.0)
            nc.sync.dma_start(wo_sb[:],
                              woT.ap().rearrange("(t p) m -> p t m", p=128))
            nc.sync.dma_start(bo_sb[:], bo.ap())

            def p2_body():
                for rt in range(ROWS_PC // 128):
                    nc.sync.dma_start(
                        ot_sb[:, :, rt * 128:(rt + 1) * 128],
                        OTs.ap()[:, rt * 128:(rt + 1) * 128]
                        .rearrange("(t p) r -> p t r", p=128))
                    ps = psp.tile([128, D], F32, tag="ps")
                    nc.tensor.matmul(ps[:], ones[:], bo_sb[:],
                                     start=True, stop=False)
                    for ft in range(FT):
                        nc.tensor.matmul(
                            ps[:], ot_sb[:, ft, rt * 128:(rt + 1) * 128],
                            wo_sb[:, ft, :],
                            start=False, stop=(ft == FT - 1))
                    r_sb = res.tile([128, D], F32, tag="r")
                    nc.scalar.copy(r_sb[:], ps[:])
                    nc.sync.dma_start(out.ap()[rt * 128:(rt + 1) * 128, :],
                                      r_sb[:])

            if reps == 1:
                p2_body()
            else:
                with tc.For_i(0, reps, 1):
                    p2_body()

    nc.compile()
    return nc


_CACHE = {}


def _get(name, builder):
    if name not in _CACHE:
        _CACHE[name] = builder()
    return _CACHE[name]


def kernel(x, attn_bias, w_in, b_in, w_out, b_out):
    x = np.asarray(x, dtype=np.float32)
    attn_bias = np.asarray(attn_bias, dtype=np.float32)
    w_in = np.asarray(w_in, dtype=np.float32)
    b_in = np.asarray(b_in, dtype=np.float32)
    w_out = np.asarray(w_out, dtype=np.float32)
    b_out = np.asarray(b_out, dtype=np.float32)

    nc1 = _get("p1", build_phase1)
    nc2 = _get("p2", build_phase2)

    xT = np.ascontiguousarray(x.reshape(ROWS, D).T)
    in_maps1 = []
    for h in range(N_CORES):
        sl_q = slice(h * DH, (h + 1) * DH)
        wqk = np.concatenate([w_in[sl_q, :] * SCALE,
                              w_in[D + h * DH:D + (h + 1) * DH, :]], axis=0)
        bqk = np.concatenate([b_in[sl_q] * SCALE,
                              b_in[D + h * DH:D + (h + 1) * DH]])
        in_maps1.append({
            "xT": xT,
            "biasT": np.ascontiguousarray(attn_bias[0, h].T),
            "wqkT": np.ascontiguousarray(wqk.T),
            "wvT": np.ascontiguousarray(
                w_in[2 * D + h * DH:2 * D + (h + 1) * DH, :].T),
            "bqk": bqk.reshape(2 * DH, 1).copy(),
            "bv": b_in[2 * D + h * DH:2 * D + (h + 1) * DH].reshape(1, DH).copy(),
        })
    res1 = run_bass_kernel_spmd(nc1, in_maps1, core_ids=list(range(N_CORES)))
    OT_full = np.concatenate([res1.results[h]["OT"] for h in range(N_CORES)],
                             axis=0)  # [512, 8192]

    woT = np.ascontiguousarray(w_out.T)
    bo = b_out.reshape(1, D).copy()
    in_maps2 = [{
        "OTs": np.ascontiguousarray(
            OT_full[:, r * ROWS_PC:(r + 1) * ROWS_PC]),
        "woT": woT,
        "bo": bo,
    } for r in range(N_CORES)]
    res2 = run_bass_kernel_spmd(nc2, in_maps2, core_ids=list(range(N_CORES)))
    out = np.concatenate([res2.results[r]["out"] for r in range(N_CORES)],
                         axis=0)
    return out.reshape(B, S, D)
